# revision 1
# baseline (speedup 1.0000x reference)
"""Trainium2 Bass kernel for nn_MultiHeadAttention_38233798869424.

Reference computation (B=4, S=2048, IN=OUT=1024, H=16, D=64):
    q = x @ Wq; k = x @ Wk; v = x @ Wv            # [B, S, 1024]
    q,k,v -> reshape(B, H, S, D)   (PLAIN view, no transpose!)
    attn per (b, h): softmax(q k^T / 8) v          # [B, H, S, D]
    ctx -> reshape(B, S, 1024); out = ctx @ Wp + bp

Key structural insight: the plain reshape means "head" h of batch b attends
only within rows [h*128, (h+1)*128) of x[b] — i.e. the problem decomposes
into B*H = 64 fully independent blocks, each a self-attention over 2048
"positions" (p = 16*r + cgroup) of dim 64 built from a [128, 1024] slice of
x. We shard the 64 blocks 8-per-core (pure SPMD, no collectives) and
compute each block's attention in a permuted position order p~ = g*128 + r
(g = column-group) which is softmax-invariant and makes every matmul
operand a contiguous/strided AP with no transposes:

  per block j (128 rows of x):
    Q~T,K~T [64(d), 2048(p~)] = (Wq|Wk col-packed)^T-chunks @ x^T-chunks
    V       [128(r), 1024]    = x-chunk @ Wv (g-th 64-col slice == V~ chunk)
    S~T[kp, qp] tiles [128, 2048] = K~T-slice.T @ Q~T      (f32r, K=64)
    expS = exp(S~T/8) (no max-subtraction needed: |s|/8 <~ 4)   [fp16]
    ctx~T_aug [65, 2048] += V_aug-chunk.T @ expS-chunk  (ones col -> sums)
    ctx~T *= 1/sums  (PE outer-product broadcast + DVE)
    out_block [128, 1024] = sum_g ctx~T[:, g-slice].T @ Wp[g-rows] + bp
"""

from contextlib import ExitStack

import numpy as np

import concourse.bass as bass

B, S, IN_DIM, OUT_DIM, H = 4, 2048, 1024, 1024, 16
D = OUT_DIM // H  # 64
N_CORES = 8
BLOCKS_PER_CORE = (B * H) // N_CORES  # 8
WAVE = 4  # blocks per wave (SBUF residency for Q/K)
N_WAVES = BLOCKS_PER_CORE // WAVE  # 2
KC = IN_DIM // 128  # 8 contraction chunks
G = 16  # column groups per block


def _build_nc():
    import concourse.tile as tile
    from concourse import bacc, mybir

    F32 = mybir.dt.float32
    F32R = mybir.dt.float32r
    F16 = mybir.dt.float16
    EXP = mybir.ActivationFunctionType.Exp

    nc = bacc.Bacc("TRN2")
    xt_d = nc.dram_tensor("xt", [128, KC * 1024], F16, kind="ExternalInput")
    wqk_d = nc.dram_tensor("wqk", [128, KC * 2048], F16, kind="ExternalInput")
    wv_d = nc.dram_tensor("wv", [128, KC * 1024], F16, kind="ExternalInput")
    wp_d = nc.dram_tensor("wp", [128, KC * 1024], F16, kind="ExternalInput")
    bp_d = nc.dram_tensor("bp", [1, 1024], F32R, kind="ExternalInput")
    ones_d = nc.dram_tensor("ones", [1, 128], F32R, kind="ExternalInput")
    out_d = nc.dram_tensor("out", [1024, 1024], F32, kind="ExternalOutput")

    with tile.TileContext(nc) as tc, ExitStack() as ctx:
        const = ctx.enter_context(tc.tile_pool(name="const", bufs=1))
        wavep = ctx.enter_context(tc.tile_pool(name="wavep", bufs=1))
        work = ctx.enter_context(tc.tile_pool(name="work", bufs=1))
        ps = ctx.enter_context(tc.tile_pool(name="ps", bufs=1, space="PSUM"))

        # chunked loads so the first matmuls can start before the whole
        # weight set has landed
        wqk_sb = const.tile([128, KC * 2048], F16)
        xt_sb = const.tile([128, KC * 1024], F16)
        wv_sb = const.tile([128, KC * 1024], F16)
        for kc in range(KC):
            nc.sync.dma_start(
                xt_sb[:, kc * 1024 : (kc + 1) * 1024],
                xt_d[:, kc * 1024 : (kc + 1) * 1024],
            )
            nc.sync.dma_start(
                wqk_sb[:, kc * 2048 : (kc + 1) * 2048],
                wqk_d[:, kc * 2048 : (kc + 1) * 2048],
            )
        for kc in range(KC):
            nc.sync.dma_start(
                wv_sb[:, kc * 1024 : (kc + 1) * 1024],
                wv_d[:, kc * 1024 : (kc + 1) * 1024],
            )
        # Wp in 128-row contraction chunks: chunk i holds rows [i*128,
        # (i+1)*128) — even g-group on partitions 0:64, odd on 64:128,
        # which is exactly what the row-packed projection pair needs.
        wp_sb = const.tile([128, KC * 1024], F16)
        nc.sync.dma_start(wp_sb, wp_d[:, :])
        bp_sb = const.tile([1, 1024], F32R)
        nc.sync.dma_start(bp_sb, bp_d[:, :])
        ones_sb = const.tile([1, 128], F32R)
        nc.sync.dma_start(ones_sb, ones_d[:, :])

        # graduated wave sizes: a tiny first wave so startup only serializes
        # one block's projections; later phase-1 work hides under the
        # previous wave's (ACT-bound) attention sweep
        wave_sizes = [4, 4]
        wave_starts = [sum(wave_sizes[:k]) for k in range(len(wave_sizes))]
        for j0, wn in zip(wave_starts, wave_sizes):
            # ---- phase 1: QKV projections for the wn blocks of this wave --
            # qk_sb free layout: j4*2048 + g*128 + r; parts 0:64 = Q~T(d),
            # parts 64:128 = K~T(d).
            qk_sb = wavep.tile([128, WAVE * 2048], F32R, tag="qk")
            # v_sb free layout: j4*1040 + g*65 + d, with an all-ones column
            # at d=64 of each g (feeds the softmax-denominator row).
            v_sb = wavep.tile([128, WAVE * 1040], F16, tag="v")
            # ones in the d=64 column of every g-group (overwritten at 0:64
            # by the V copies below) -> softmax denominator row of ctx~T.
            nc.vector.memset(v_sb, 1.0)

            for g in range(G):
                qk_ps = ps.tile([128, wn * 128], F32, tag="s", bufs=2)
                for kc in range(KC):
                    nc.tensor.matmul(
                        qk_ps,
                        lhsT=wqk_sb[:, kc * 2048 + g * 128 : kc * 2048 + g * 128 + 128],
                        rhs=xt_sb[:, kc * 1024 + j0 * 128 : kc * 1024 + (j0 + wn) * 128],
                        start=(kc == 0),
                        stop=(kc == KC - 1),
                    )
                # scatter [128, (j4)(r)] -> qk_sb[:, j4*2048 + g*128 + r]
                out_view = qk_sb.rearrange("p (j f) -> p j f", j=WAVE)[
                    :, 0:wn, g * 128 : g * 128 + 128
                ]
                in_view = qk_ps.rearrange("p (j f) -> p j f", j=wn)
                nc.vector.tensor_copy(out_view, in_view)

            for j4 in range(wn):
                for ns in range(2):
                    v_ps = ps.tile([128, 512], F32, tag="s", bufs=2)
                    for kc in range(KC):
                        nc.tensor.matmul(
                            v_ps,
                            lhsT=xt_sb[
                                :,
                                kc * 1024 + (j0 + j4) * 128 : kc * 1024
                                + (j0 + j4) * 128
                                + 128,
                            ],
                            rhs=wv_sb[:, kc * 1024 + ns * 512 : kc * 1024 + ns * 512 + 512],
                            start=(kc == 0),
                            stop=(kc == KC - 1),
                        )
                    # [128, (8g)(64d)] -> v_sb[:, j4*1040 + (ns*8+g)*65 + d]
                    o = v_sb.rearrange("p (a e) -> p a e", e=65)[
                        :, j4 * 16 + ns * 8 : j4 * 16 + ns * 8 + 8, 0:64
                    ]
                    nc.vector.tensor_copy(o, v_ps.rearrange("p (a e) -> p a e", e=64))

            # ---- phase 2: attention per block ----
            for j4 in range(wn):
                j = j0 + j4
                # k2: K~T chunk pairs packed on complementary partition
                # halves — even kpos-chunks on 0:64, odd on 64:128 — so two
                # K=64 scores matmuls run concurrently via row tiling.
                k2 = work.tile([128, 1024], F32R, tag="k2", bufs=2)
                ksrc = qk_sb[64:128, j4 * 2048 : j4 * 2048 + 2048].rearrange(
                    "p (i two r) -> p i two r", two=2, r=128
                )
                k2lo = k2[0:64, :].rearrange("p (i r) -> p i r", r=128)
                k2hi = k2[64:128, :].rearrange("p (i r) -> p i r", r=128)
                nc.sync.dma_start(k2lo, ksrc[:, :, 0, :])
                nc.sync.dma_start(k2hi, ksrc[:, :, 1, :])
                # Q~T duplicated onto partitions 64:128 (rhs of the B-side)
                qq = work.tile([128, 2048], F32R, tag="qq", bufs=1)
                nc.sync.dma_start(
                    qq[64:128, :], qk_sb[0:64, j4 * 2048 : j4 * 2048 + 2048]
                )

                ctxT_sb = work.tile([128, 2048], F16, tag="ctxT", bufs=2)
                for h in range(2):
                    # per-half ctx accumulator: [65, 1024] = 2 PSUM banks so
                    # halves/blocks can overlap (tag "c" also holds psA)
                    ctx_ps = ps.tile([65, 1024], F32, tag="c", bufs=2)
                    for i in range(G // 2):
                        sA = ps.tile([128, 1024], F32, tag="s", bufs=2)
                        sB = ps.tile([128, 1024], F32, tag="s", bufs=2)
                        for ns in range(2):
                            q_off = j4 * 2048 + h * 1024 + ns * 512
                            nc.tensor.matmul(
                                sA[:, ns * 512 : ns * 512 + 512],
                                lhsT=k2[0:64, i * 128 : i * 128 + 128],
                                rhs=qk_sb[0:64, q_off : q_off + 512],
                                start=True,
                                stop=True,
                                tile_position=(0, 0),
                            )
                            q_off2 = h * 1024 + ns * 512
                            nc.tensor.matmul(
                                sB[:, ns * 512 : ns * 512 + 512],
                                lhsT=k2[64:128, i * 128 : i * 128 + 128],
                                rhs=qq[64:128, q_off2 : q_off2 + 512],
                                start=True,
                                stop=True,
                                tile_position=(64, 0),
                            )
                        esA = work.tile([128, 1024], F16, tag="es", bufs=6)
                        nc.scalar.activation(esA, sA, EXP, scale=0.125)
                        esB = work.tile([128, 1024], F16, tag="es", bufs=6)
                        nc.scalar.activation(esB, sB, EXP, scale=0.125)
                        for gk, es in ((2 * i, esA), (2 * i + 1, esB)):
                            for ns in range(2):
                                nc.tensor.matmul(
                                    ctx_ps[:, ns * 512 : ns * 512 + 512],
                                    lhsT=v_sb[
                                        :,
                                        j4 * 1040 + gk * 65 : j4 * 1040 + gk * 65 + 65,
                                    ],
                                    rhs=es[:, ns * 512 : ns * 512 + 512],
                                    start=(gk == 0),
                                    stop=(gk == G - 1),
                                )
                    # normalize: 1/sums (row 64), gpsimd partition-broadcast,
                    # one DVE multiply into fp16 ctx~T
                    inv_sb = work.tile([1, 1024], F32, tag="inv", bufs=2)
                    nc.vector.reciprocal(inv_sb, ctx_ps[64:65, :])
                    invb = work.tile([64, 1024], F32, tag="invb", bufs=2)
                    nc.gpsimd.partition_broadcast(invb, inv_sb)
                    nc.vector.tensor_mul(
                        ctxT_sb[0:64, h * 1024 : h * 1024 + 1024],
                        ctx_ps[0:64, :],
                        invb,
                    )
                    # duplicate ctx~T onto partitions 64:128 for the
                    # row-packed projection pair
                    nc.sync.dma_start(
                        ctxT_sb[64:128, h * 1024 : h * 1024 + 1024],
                        ctxT_sb[0:64, h * 1024 : h * 1024 + 1024],
                    )

                # final projection: row-packed pairs — even contraction
                # chunks accumulate in psA (+bias), odd in psB, added on DVE
                psA = ps.tile([128, 1024], F32, tag="c", bufs=2)
                psB = ps.tile([128, 1024], F32, tag="s", bufs=2)
                for i in range(KC):
                    for ns in range(2):
                        nc.tensor.matmul(
                            psA[:, ns * 512 : ns * 512 + 512],
                            lhsT=ctxT_sb[0:64, (2 * i) * 128 : (2 * i) * 128 + 128],
                            rhs=wp_sb[0:64, i * 1024 + ns * 512 : i * 1024 + ns * 512 + 512],
                            start=(i == 0),
                            stop=False,
                            tile_position=(0, 0),
                        )
                        nc.tensor.matmul(
                            psB[:, ns * 512 : ns * 512 + 512],
                            lhsT=ctxT_sb[64:128, (2 * i + 1) * 128 : (2 * i + 1) * 128 + 128],
                            rhs=wp_sb[64:128, i * 1024 + ns * 512 : i * 1024 + ns * 512 + 512],
                            start=(i == 0),
                            stop=(i == KC - 1),
                            tile_position=(64, 0),
                        )
                for ns in range(2):
                    nc.tensor.matmul(
                        psA[:, ns * 512 : ns * 512 + 512],
                        lhsT=ones_sb[:, 0:128],
                        rhs=bp_sb[:, ns * 512 : ns * 512 + 512],
                        start=False,
                        stop=True,
                    )
                obf = work.tile([128, 1024], F32, tag="obf", bufs=2)
                nc.vector.tensor_copy(obf, psB)
                out_sb = work.tile([128, 1024], F32, tag="outsb", bufs=2)
                nc.vector.tensor_add(out_sb, psA, obf)
                nc.sync.dma_start(out_d[j * 128 : j * 128 + 128, :], out_sb)

    nc.compile()
    return nc


_compiled = {}


def kernel(x, Wq, Wk, Wv, Wp, bp):
    from concourse.bass_utils import run_bass_kernel_spmd

    x = np.asarray(x, dtype=np.float32)
    Wq = np.asarray(Wq, dtype=np.float32)
    Wk = np.asarray(Wk, dtype=np.float32)
    Wv = np.asarray(Wv, dtype=np.float32)
    Wp = np.asarray(Wp, dtype=np.float32)
    bp = np.asarray(bp, dtype=np.float32)

    f16 = np.float16

    # weights, shared by all cores
    wqk = np.empty((IN_DIM, G, 128), np.float32)
    wqk[:, :, :64] = Wq.reshape(IN_DIM, G, 64)
    wqk[:, :, 64:] = Wk.reshape(IN_DIM, G, 64)
    wqk_sb = (
        wqk.reshape(KC, 128, 2048).transpose(1, 0, 2).reshape(128, KC * 2048)
    ).astype(f16)
    wv_sb = (
        Wv.reshape(KC, 128, 1024).transpose(1, 0, 2).reshape(128, KC * 1024)
    ).astype(f16)
    wp_sb = (
        Wp.reshape(KC, 128, 1024).transpose(1, 0, 2).reshape(128, KC * 1024)
    ).astype(f16)
    bp_sb = bp.reshape(1, 1024).astype(np.float32)

    x_flat = x.reshape(B * S, IN_DIM)
    in_maps = []
    for c in range(N_CORES):
        slab = x_flat[c * 1024 : (c + 1) * 1024]  # [1024 rows, 1024 k]
        xt = np.ascontiguousarray(slab.T)  # [k, jr]
        xt_sb = (
            xt.reshape(KC, 128, 1024).transpose(1, 0, 2).reshape(128, KC * 1024)
        ).astype(f16)
        in_maps.append(
            {
                "xt": xt_sb,
                "wqk": wqk_sb,
                "wv": wv_sb,
                "wp": wp_sb,
                "bp": bp_sb,
                "ones": np.ones((1, 128), np.float32),
            }
        )

    if "nc" not in _compiled:
        _compiled["nc"] = _build_nc()
    nc = _compiled["nc"]

    res = run_bass_kernel_spmd(nc, in_maps, list(range(N_CORES)))

    out = np.empty((B * S, OUT_DIM), np.float32)
    for c in range(N_CORES):
        out[c * 1024 : (c + 1) * 1024] = res.results[c]["out"]
    return out.reshape(B, S, OUT_DIM)

